# revision 14
# baseline (speedup 1.0000x reference)
"""Trainium2 Bass kernel for nn_Attention_5609227288590 (sparse_attention).

Math: the reference's suppress branch with THRES=1.0 has an all-True mask
(every attn value <= its row max), so it reduces exactly to

    attn' = suppress * attn^2 / (rowsum(attn) + 1e-6)

with rowsum(attn) == 1 up to fp rounding of the softmax itself.  Writing
P = exp(S) (no max subtraction needed: |S| <= ~4 for this distribution),
Z_i = sum_j P_ij:

    out_head[i, :] = c * (P∘P) @ V / Z_i^2 ,   c = suppress / (1 + 1e-6)

Per-core layout (data-parallel over batch, 2 batches/core):
  - qkT (channels x tokens) computed with w_qkv as stationary, x^T as moving
    -> Q^T/K^T land head-pair-stacked on partitions for the row-packed
    S^T = K^T.T @ Q^T matmuls (contraction d=64, 2 heads share the PE array).
    The softmax scale is folded into the Q weights host-side so Exp runs
    with the default scale.
  - V computed in (tokens x channels) layout -> V slices are direct lhsT for
    the PV matmul; P2^T is the moving operand (contraction j on partitions).
  - Z by ones[128,64]-stationary matmuls col-packed 2 heads/bank, giving Z
    broadcast across 64 partitions, matching the PV psum layout.
  - Exp(S) is the ONLY ScalarE work (the per-jt critical path, ~2.2us/jt);
    P^2 is one fused [128,2048] DVE square per jt, and 1/Z^2 is DVE
    reciprocal_approx_fast + two multiplies, ordered so psum_z/psum_o
    release as early as possible for the next pair-batch.
  - Attention runs as a single global slot pipeline over (pair, batch, jt);
    Z/PV matmuls lag the S/Exp/square stream by LAG slots (deep pt/p2t
    pools) so the in-order PE queue never blocks the exp cadence, including
    across pair-batch boundaries.
  - attn_outT (channels x tokens) feeds out-proj as lhsT directly; bias is
    added via a K=1 matmul with an all-ones stationary.
  - Input DMAs are ordered (first QK weight block, then x chunks c-major,
    then remaining weights) so the first projection matmul issues ~3us in.
"""

import numpy as np
import ml_dtypes

import concourse.bass as bass
import concourse.mybir as mybir
import concourse.tile as tile
from concourse import bacc
from concourse.bass_utils import run_bass_kernel_spmd

BF16 = mybir.dt.bfloat16
F32 = mybir.dt.float32
AF = mybir.ActivationFunctionType

N_CORES = 8
B = 16
N = 1024
DIM = 768
HEADS = 12
DH = 64
B_PC = B // N_CORES          # 2 batches per core
T = B_PC * N                 # 2048 tokens per core
PAIRS = HEADS // 2           # 6 head pairs
KT = DIM // 128              # 6 contraction tiles for projections
SCALE = DH ** -0.5           # 0.125
JTS = N // 128               # 8 j-tiles per attention step

LAST_RESULTS = None  # BassKernelResults of the last run (for test.py)


def _patch_act_tables():
    """Pin all activations to the natural_log_exp_and_others table set.

    The kernel only uses Exp and Ln.  Left alone, bacc assigns Exp and Ln to
    different sets and the inner loop thrashes ACT_TABLE_LOAD (~2.7us each).
    Emptying every other set (order preserved, so set ids stay valid) forces
    one load total.
    """
    import concourse.hw_specs as hw_specs

    if getattr(bacc, "_act_tables_patched", False):
        return
    orig = hw_specs.get_activation_tables

    def patched(module_arch):
        tabs = orig(module_arch)
        return {
            name: (funcs if name == "natural_log_exp_and_others" else set())
            for name, funcs in tabs.items()
        }

    bacc.get_activation_tables = patched
    bacc._act_tables_patched = True


def _build_kernel():
    _patch_act_tables()
    nc = bacc.Bacc("TRN2", target_bir_lowering=False, debug=False)

    xT = nc.dram_tensor("xT", [DIM, T], BF16, kind="ExternalInput")
    w_qk = nc.dram_tensor("w_qk", [DIM, 2 * DIM], BF16, kind="ExternalInput")
    w_v = nc.dram_tensor("w_v", [DIM, DIM], BF16, kind="ExternalInput")
    w_out = nc.dram_tensor("w_out", [DIM, DIM], BF16, kind="ExternalInput")
    b_out = nc.dram_tensor("b_out", [1, DIM], BF16, kind="ExternalInput")
    out = nc.dram_tensor("out", [T, DIM], F32, kind="ExternalOutput")

    with tile.TileContext(nc) as tc:
        _body(nc, tc, xT, w_qk, w_v, w_out, b_out, out)
    nc.compile()
    return nc


def _body(nc, tc, xT, w_qk, w_v, w_out, b_out, out):
    from contextlib import ExitStack

    ctx = ExitStack()
    with ctx:
        singles = ctx.enter_context(tc.tile_pool(name="singles", bufs=1))

        # ---- persistent SBUF tensors ----
        w_qk_sb = singles.tile([128, KT, 2 * DIM], BF16)
        w_v_sb = singles.tile([128, KT, DIM], BF16)
        w_out_sb = singles.tile([128, KT, DIM], BF16)
        b_out_sb = singles.tile([1, DIM], BF16)
        ones64 = singles.tile([128, DH], BF16)
        ones1 = singles.tile([1, 128], BF16)
        qkT_sb = singles.tile([128, 2 * PAIRS, T], BF16)   # tiles 0-5 Q, 6-11 K
        v_sb = singles.tile([128, T // 128, DIM], BF16)    # [t, c] layout
        aoT_sb = singles.tile([128, KT, T], BF16)          # attn-outT stacked

        w_qk_r = w_qk.rearrange("(ko p) c -> p ko c", p=128)
        w_v_r = w_v.rearrange("(ko p) c -> p ko c", p=128)
        xt_r = xT.rearrange("(ko p) t -> p ko t", p=128)

        # DMA order: first QK weight chunk, then x (c-major so the c-outer
        # projection loop can start as chunks land), then remaining weights.
        nc.sync.dma_start(w_qk_sb[:, :, 0:128], w_qk_r[:, :, 0:128])

        # ---- phase 1+2: projections ----
        with (
            tc.tile_pool(name="xt_pool", bufs=1) as xt_pool,
            tc.tile_pool(name="proj_ps", bufs=2, space="PSUM") as proj_ps,
        ):
            xt_sb = xt_pool.tile([128, KT, T], BF16)
            for c in range(T // 512):
                for kt in range(KT):
                    nc.sync.dma_start(
                        xt_sb[:, kt, c * 512:(c + 1) * 512],
                        xt_r[:, kt, c * 512:(c + 1) * 512],
                    )
            for mt in range(1, 2 * PAIRS):
                nc.sync.dma_start(
                    w_qk_sb[:, :, mt * 128:(mt + 1) * 128],
                    w_qk_r[:, :, mt * 128:(mt + 1) * 128],
                )
            for kt in range(KT):
                nc.sync.dma_start(w_v_sb[:, kt], w_v_r[:, kt])
            nc.sync.dma_start(
                w_out_sb, w_out.rearrange("(ko p) c -> p ko c", p=128)
            )
            nc.sync.dma_start(b_out_sb, b_out[:, :])
            nc.any.memset(ones64, 1.0)
            nc.any.memset(ones1, 1.0)

            for mt in range(2 * PAIRS):
                ps = proj_ps.tile([128, T], F32, tag="proj")
                for c in range(T // 512):
                    for kt in range(KT):
                        nc.tensor.matmul(
                            ps[:, c * 512:(c + 1) * 512],
                            w_qk_sb[:, kt, mt * 128:(mt + 1) * 128],
                            xt_sb[:, kt, c * 512:(c + 1) * 512],
                            start=(kt == 0),
                            stop=(kt == KT - 1),
                        )
                nc.vector.tensor_copy(out=qkT_sb[:, mt, :], in_=ps)

            for mt in range(T // 128):
                ps = proj_ps.tile([128, DIM], F32, tag="proj")
                for kt in range(KT):
                    for c0, c1 in ((0, 512), (512, 768)):
                        nc.tensor.matmul(
                            ps[:, c0:c1],
                            xt_sb[:, kt, mt * 128:(mt + 1) * 128],
                            w_v_sb[:, kt, c0:c1],
                            start=(kt == 0),
                            stop=(kt == KT - 1),
                        )
                nc.vector.tensor_copy(out=v_sb[:, mt, :], in_=ps)

        # ---- phase 3: attention, per (head pair, batch) ----
        with (
            tc.tile_pool(name="pt_pool", bufs=6) as pt_pool,
            tc.tile_pool(name="p2t_pool", bufs=6) as p2t_pool,
            tc.tile_pool(name="z_sb_pool", bufs=2) as z_sb_pool,
            tc.tile_pool(name="s_ps", bufs=2, space="PSUM") as s_ps,
            tc.tile_pool(name="o_ps", bufs=1, space="PSUM") as o_ps,
            tc.tile_pool(name="z_ps", bufs=1, space="PSUM") as z_ps,
        ):
            pts, p2ts = {}, {}
            obank = {}  # (h, b) -> (psum_o, psum_z)

            def s_exp_sq(h, b, jt):
                t0 = b * N
                qT = qkT_sb[:, h, t0:t0 + N]
                kT_ = qkT_sb[:, PAIRS + h, t0:t0 + N]
                pt = pt_pool.tile([128, 2 * N], BF16, tag="pt")
                p2t = p2t_pool.tile([128, 2 * N], BF16, tag="p2t")
                pts[(h, b, jt)], p2ts[(h, b, jt)] = pt, p2t
                with tc.high_priority(offset=90):
                    # hh-grouped: h1's matmuls launch right after the prior
                    # ACT(h1) frees its psum tile and hide under ACT(h0)
                    for hh in (1, 0):
                        d0, d1 = hh * 64, hh * 64 + 64
                        ps = s_ps.tile([128, N], F32, tag="s", name="psS")
                        for c in range(2):
                            nc.tensor.matmul(
                                ps[:, c * 512:(c + 1) * 512],
                                kT_[d0:d1, jt * 128:(jt + 1) * 128],
                                qT[d0:d1, c * 512:(c + 1) * 512],
                                start=True,
                                stop=True,
                            )
                        nc.scalar.activation(
                            pt[:, hh * N:(hh + 1) * N], ps, AF.Exp,
                        )
                    # one fused square for both head halves (DVE 2x bf16)
                    nc.vector.tensor_mul(out=p2t, in0=pt, in1=pt)

            def zpv(h, b, jt):
                # skip_group_check: the sim's global zero-region check
                # mishandles base_partition!=0; col-split groups are
                # HW-safe (verified by direct probe).
                if jt == 0:
                    obank[(h, b)] = (
                        o_ps.tile([128, 1024], F32, tag="o", name="psum_o"),
                        z_ps.tile([128, 1024], F32, tag="z", name="psum_z"),
                    )
                psum_o, psum_z = obank[(h, b)]
                pt, p2t = pts.pop((h, b, jt)), p2ts.pop((h, b, jt))
                vt = v_sb[:, b * 8 + jt, :]
                first, last = jt == 0, jt == JTS - 1
                for hh in (1, 0):
                    d0, d1 = hh * 64, hh * 64 + 64
                    ch0 = h * 128 + hh * 64
                    for c in range(2):
                        nc.tensor.matmul(
                            psum_z[d0:d1, c * 512:(c + 1) * 512],
                            ones64,
                            pt[:, hh * N + c * 512:hh * N + (c + 1) * 512],
                            start=first,
                            stop=last,
                            skip_group_check=True,
                        )
                    for c in range(2):
                        nc.tensor.matmul(
                            psum_o[d0:d1, c * 512:(c + 1) * 512],
                            vt[:, ch0:ch0 + 64],
                            p2t[:, hh * N + c * 512:hh * N + (c + 1) * 512],
                            start=first,
                            stop=last,
                            skip_group_check=True,
                        )
                if last:
                    # 1/Z^2 on DVE as (psum_o * zinv) * zinv: psum_o is
                    # released after the first multiply, shortening the wait
                    # of the next pair-batch's PV matmuls.
                    psum_o, psum_z = obank.pop((h, b))
                    zinv = z_sb_pool.tile([128, 1024], F32, tag="zinv")
                    otmp = z_sb_pool.tile([128, 1024], F32, tag="otmp")
                    nc.vector.reciprocal_approx_fast(zinv, psum_z)
                    nc.vector.tensor_mul(out=otmp, in0=psum_o, in1=zinv)
                    nc.vector.tensor_mul(
                        out=aoT_sb[:, h, b * N:(b + 1) * N],
                        in0=otmp,
                        in1=zinv,
                    )

            # global slot pipeline: zpv lags s_exp_sq by LAG slots so the
            # PE queue never blocks on the exp/square of the same slot and
            # pair-batch boundaries stay full.
            LAG = 2
            slots = [
                (h, b, jt)
                for h in range(PAIRS)
                for b in range(B_PC)
                for jt in range(JTS)
            ]
            for k in range(len(slots) + LAG):
                if k < len(slots):
                    s_exp_sq(*slots[k])
                if k >= LAG:
                    zpv(*slots[k - LAG])

        # ---- phase 4: out projection + bias ----
        with (
            tc.tile_pool(name="f_sb", bufs=3) as f_sb,
            tc.tile_pool(name="f_ps", bufs=2, space="PSUM") as f_ps,
        ):
            for mt in range(T // 128):
                ps = f_ps.tile([128, DIM], F32, tag="f")
                for c0, c1 in ((0, 512), (512, 768)):
                    for kt in range(KT):
                        nc.tensor.matmul(
                            ps[:, c0:c1],
                            aoT_sb[:, kt, mt * 128:(mt + 1) * 128],
                            w_out_sb[:, kt, c0:c1],
                            start=(kt == 0),
                            stop=False,
                        )
                    nc.tensor.matmul(
                        ps[:, c0:c1],
                        ones1[0:1, 0:128],
                        b_out_sb[0:1, c0:c1],
                        start=False,
                        stop=True,
                    )
                o_sb = f_sb.tile([128, DIM], F32, tag="fo")
                nc.vector.tensor_copy(out=o_sb, in_=ps)
                nc.sync.dma_start(out[mt * 128:(mt + 1) * 128, :], o_sb)


def _ensure_ntff_hook():
    """Install the NTFF profiling hook that bass_utils expects under axon.

    This agent image's ``antenv`` lacks ``axon_hooks``; replicate the shim
    trn_boot would install, backed by /opt/axon/libaxon_pjrt.so.
    """
    import sys
    import types

    try:
        from antenv.axon_hooks import get_axon_ntff_profile_hook  # noqa: F401

        return
    except ImportError:
        pass
    import antenv

    mod = types.ModuleType("antenv.axon_hooks")
    _hook = [None]
    mod.set_axon_ntff_profile_hook = lambda h: _hook.__setitem__(0, h)
    mod.get_axon_ntff_profile_hook = lambda: _hook[0]
    sys.modules["antenv.axon_hooks"] = mod
    antenv.axon_hooks = mod
    try:
        from trn_agent_boot.trn_boot import _ntff_profile_via_ctypes

        mod.set_axon_ntff_profile_hook(
            _ntff_profile_via_ctypes("/opt/axon/libaxon_pjrt.so")
        )
    except Exception:
        pass


_NC_CACHE = None


def _get_nc():
    global _NC_CACHE
    if _NC_CACHE is None:
        _NC_CACHE = _build_kernel()
    return _NC_CACHE


def kernel(x, w_qkv, w_out, b_out, suppress, _trace=False):
    global LAST_RESULTS
    x = np.asarray(x, dtype=np.float32)
    w_qkv = np.asarray(w_qkv, dtype=np.float32)
    w_out_np = np.asarray(w_out, dtype=np.float32)
    b_out_np = np.asarray(b_out, dtype=np.float32)
    c = float(np.asarray(suppress)) / (1.0 + 1e-6)

    bf = ml_dtypes.bfloat16
    w_qk_f = np.ascontiguousarray(w_qkv[:, : 2 * DIM]).copy()
    w_qk_f[:, :DIM] *= SCALE  # fold softmax scale into Q so Exp runs scale-free
    w_qk_b = w_qk_f.astype(bf)
    w_v_b = np.ascontiguousarray(w_qkv[:, 2 * DIM:] * c).astype(bf)
    w_out_b = w_out_np.astype(bf)
    b_out_b = b_out_np.reshape(1, DIM).astype(bf)

    nc = _get_nc()
    in_maps = []
    for core in range(N_CORES):
        xs = x[core * B_PC:(core + 1) * B_PC].reshape(T, DIM)
        xT_b = np.ascontiguousarray(xs.T).astype(bf)
        in_maps.append(
            {
                "xT": xT_b,
                "w_qk": w_qk_b,
                "w_v": w_v_b,
                "w_out": w_out_b,
                "b_out": b_out_b,
            }
        )

    if _trace:
        _ensure_ntff_hook()
    res = run_bass_kernel_spmd(
        nc, in_maps, core_ids=list(range(N_CORES)), trace=_trace
    )
    LAST_RESULTS = res
    outs = [res.results[cc]["out"].reshape(B_PC, N, DIM) for cc in range(N_CORES)]
    return np.concatenate(outs, axis=0)



# revision 15
# speedup vs baseline: 1.0170x; 1.0170x over previous
"""Trainium2 Bass kernel for nn_Attention_5609227288590 (sparse_attention).

Math: the reference's suppress branch with THRES=1.0 has an all-True mask
(every attn value <= its row max), so it reduces exactly to

    attn' = suppress * attn^2 / (rowsum(attn) + 1e-6)

with rowsum(attn) == 1 up to fp rounding of the softmax itself.  Writing
P = exp(S) (no max subtraction needed: |S| <= ~4 for this distribution),
Z_i = sum_j P_ij:

    out_head[i, :] = c * (P∘P) @ V / Z_i^2 ,   c = suppress / (1 + 1e-6)

Per-core layout (data-parallel over batch, 2 batches/core):
  - qkT (channels x tokens) computed with w_qkv as stationary, x^T as moving
    -> Q^T/K^T land head-pair-stacked on partitions for the row-packed
    S^T = K^T.T @ Q^T matmuls (contraction d=64, 2 heads share the PE array).
    The softmax scale is folded into the Q weights host-side so Exp runs
    with the default scale.
  - V computed in (tokens x channels) layout -> V slices are direct lhsT for
    the PV matmul; P2^T is the moving operand (contraction j on partitions).
  - Z by ones[128,64]-stationary matmuls col-packed 2 heads/bank, giving Z
    broadcast across 64 partitions, matching the PV psum layout.
  - Exp(S) is the ONLY ScalarE work (the per-jt critical path, ~2.2us/jt);
    P^2 is one fused [128,2048] DVE square per jt, and 1/Z^2 is DVE
    reciprocal_approx_fast + two multiplies, ordered so psum_z/psum_o
    release as early as possible for the next pair-batch.
  - Attention runs as a single global slot pipeline over (pair, batch, jt);
    Z/PV matmuls lag the S/Exp/square stream by LAG slots (deep pt/p2t
    pools) so the in-order PE queue never blocks the exp cadence, including
    across pair-batch boundaries.
  - attn_outT (channels x tokens) feeds out-proj as lhsT directly; bias is
    added via a K=1 matmul with an all-ones stationary.
  - Input DMAs are ordered (first QK weight block, then x chunks c-major,
    then remaining weights) so the first projection matmul issues ~3us in.
"""

import numpy as np
import ml_dtypes

import concourse.bass as bass
import concourse.mybir as mybir
import concourse.tile as tile
from concourse import bacc
from concourse.bass_utils import run_bass_kernel_spmd

BF16 = mybir.dt.bfloat16
F32 = mybir.dt.float32
AF = mybir.ActivationFunctionType

N_CORES = 8
B = 16
N = 1024
DIM = 768
HEADS = 12
DH = 64
B_PC = B // N_CORES          # 2 batches per core
T = B_PC * N                 # 2048 tokens per core
PAIRS = HEADS // 2           # 6 head pairs
KT = DIM // 128              # 6 contraction tiles for projections
SCALE = DH ** -0.5           # 0.125
JTS = N // 128               # 8 j-tiles per attention step

LAST_RESULTS = None  # BassKernelResults of the last run (for test.py)


def _patch_act_tables():
    """Pin all activations to the natural_log_exp_and_others table set.

    The kernel only uses Exp and Ln.  Left alone, bacc assigns Exp and Ln to
    different sets and the inner loop thrashes ACT_TABLE_LOAD (~2.7us each).
    Emptying every other set (order preserved, so set ids stay valid) forces
    one load total.
    """
    import concourse.hw_specs as hw_specs

    if getattr(bacc, "_act_tables_patched", False):
        return
    orig = hw_specs.get_activation_tables

    def patched(module_arch):
        tabs = orig(module_arch)
        return {
            name: (funcs if name == "natural_log_exp_and_others" else set())
            for name, funcs in tabs.items()
        }

    bacc.get_activation_tables = patched
    bacc._act_tables_patched = True


def _build_kernel():
    _patch_act_tables()
    nc = bacc.Bacc("TRN2", target_bir_lowering=False, debug=False)

    xT = nc.dram_tensor("xT", [DIM, T], BF16, kind="ExternalInput")
    w_qk = nc.dram_tensor("w_qk", [DIM, 2 * DIM], BF16, kind="ExternalInput")
    w_v = nc.dram_tensor("w_v", [DIM, DIM], BF16, kind="ExternalInput")
    w_out = nc.dram_tensor("w_out", [DIM, DIM], BF16, kind="ExternalInput")
    b_out = nc.dram_tensor("b_out", [1, DIM], BF16, kind="ExternalInput")
    out = nc.dram_tensor("out", [T, DIM], F32, kind="ExternalOutput")

    with tile.TileContext(nc) as tc:
        _body(nc, tc, xT, w_qk, w_v, w_out, b_out, out)
    nc.compile()
    return nc


def _body(nc, tc, xT, w_qk, w_v, w_out, b_out, out):
    from contextlib import ExitStack

    ctx = ExitStack()
    with ctx:
        singles = ctx.enter_context(tc.tile_pool(name="singles", bufs=1))

        # ---- persistent SBUF tensors ----
        w_qk_sb = singles.tile([128, KT, 2 * DIM], BF16)
        w_v_sb = singles.tile([128, KT, DIM], BF16)
        w_out_sb = singles.tile([128, KT, DIM], BF16)
        b_out_sb = singles.tile([1, DIM], BF16)
        ones64 = singles.tile([128, DH], BF16)
        ones1 = singles.tile([1, 128], BF16)
        qkT_sb = singles.tile([128, 2 * PAIRS, T], BF16)   # tiles 0-5 Q, 6-11 K
        v_sb = singles.tile([128, T // 128, DIM], BF16)    # [t, c] layout
        aoT_sb = singles.tile([128, KT, T], BF16)          # attn-outT stacked

        w_qk_r = w_qk.rearrange("(ko p) c -> p ko c", p=128)
        w_v_r = w_v.rearrange("(ko p) c -> p ko c", p=128)
        xt_r = xT.rearrange("(ko p) t -> p ko t", p=128)

        # DMA order: first QK weight chunk, then x (c-major so the c-outer
        # projection loop can start as chunks land), then remaining weights.
        nc.sync.dma_start(w_qk_sb[:, :, 0:128], w_qk_r[:, :, 0:128])

        # ---- phase 1+2: projections ----
        with (
            tc.tile_pool(name="xt_pool", bufs=1) as xt_pool,
            tc.tile_pool(name="proj_ps", bufs=2, space="PSUM") as proj_ps,
        ):
            xt_sb = xt_pool.tile([128, KT, T], BF16)
            for c in range(T // 512):
                for kt in range(KT):
                    nc.sync.dma_start(
                        xt_sb[:, kt, c * 512:(c + 1) * 512],
                        xt_r[:, kt, c * 512:(c + 1) * 512],
                    )
            for mt in range(1, 2 * PAIRS):
                nc.sync.dma_start(
                    w_qk_sb[:, :, mt * 128:(mt + 1) * 128],
                    w_qk_r[:, :, mt * 128:(mt + 1) * 128],
                )
            for kt in range(KT):
                nc.sync.dma_start(w_v_sb[:, kt], w_v_r[:, kt])
            nc.sync.dma_start(
                w_out_sb, w_out.rearrange("(ko p) c -> p ko c", p=128)
            )
            nc.sync.dma_start(b_out_sb, b_out[:, :])
            nc.any.memset(ones64, 1.0)
            nc.any.memset(ones1, 1.0)

            for mt in range(2 * PAIRS):
                ps = proj_ps.tile([128, T], F32, tag="proj")
                for c in range(T // 512):
                    for kt in range(KT):
                        nc.tensor.matmul(
                            ps[:, c * 512:(c + 1) * 512],
                            w_qk_sb[:, kt, mt * 128:(mt + 1) * 128],
                            xt_sb[:, kt, c * 512:(c + 1) * 512],
                            start=(kt == 0),
                            stop=(kt == KT - 1),
                        )
                nc.vector.tensor_copy(out=qkT_sb[:, mt, :], in_=ps)

            for mt in range(T // 128):
                ps = proj_ps.tile([128, DIM], F32, tag="proj")
                for kt in range(KT):
                    for c0, c1 in ((0, 512), (512, 768)):
                        nc.tensor.matmul(
                            ps[:, c0:c1],
                            xt_sb[:, kt, mt * 128:(mt + 1) * 128],
                            w_v_sb[:, kt, c0:c1],
                            start=(kt == 0),
                            stop=(kt == KT - 1),
                        )
                nc.vector.tensor_copy(out=v_sb[:, mt, :], in_=ps)

        # ---- phase 3: attention, per (head pair, batch) ----
        with (
            tc.tile_pool(name="pt_pool", bufs=7) as pt_pool,
            tc.tile_pool(name="p2t_pool", bufs=7) as p2t_pool,
            tc.tile_pool(name="z_sb_pool", bufs=2) as z_sb_pool,
            tc.tile_pool(name="s_ps", bufs=2, space="PSUM") as s_ps,
            tc.tile_pool(name="o_ps", bufs=1, space="PSUM") as o_ps,
            tc.tile_pool(name="z_ps", bufs=1, space="PSUM") as z_ps,
        ):
            pts, p2ts = {}, {}
            obank = {}  # (h, b) -> (psum_o, psum_z)

            def s_exp_sq(h, b, jt):
                t0 = b * N
                qT = qkT_sb[:, h, t0:t0 + N]
                kT_ = qkT_sb[:, PAIRS + h, t0:t0 + N]
                pt = pt_pool.tile([128, 2 * N], BF16, tag="pt")
                p2t = p2t_pool.tile([128, 2 * N], BF16, tag="p2t")
                pts[(h, b, jt)], p2ts[(h, b, jt)] = pt, p2t
                with tc.high_priority(offset=90):
                    # hh-grouped: h1's matmuls launch right after the prior
                    # ACT(h1) frees its psum tile and hide under ACT(h0)
                    for hh in (1, 0):
                        d0, d1 = hh * 64, hh * 64 + 64
                        ps = s_ps.tile([128, N], F32, tag="s", name="psS")
                        for c in range(2):
                            nc.tensor.matmul(
                                ps[:, c * 512:(c + 1) * 512],
                                kT_[d0:d1, jt * 128:(jt + 1) * 128],
                                qT[d0:d1, c * 512:(c + 1) * 512],
                                start=True,
                                stop=True,
                            )
                        nc.scalar.activation(
                            pt[:, hh * N:(hh + 1) * N], ps, AF.Exp,
                        )
                    # one fused square for both head halves (DVE 2x bf16)
                    nc.vector.tensor_mul(out=p2t, in0=pt, in1=pt)

            def zpv(h, b, jt):
                # skip_group_check: the sim's global zero-region check
                # mishandles base_partition!=0; col-split groups are
                # HW-safe (verified by direct probe).
                if jt == 0:
                    obank[(h, b)] = (
                        o_ps.tile([128, 1024], F32, tag="o", name="psum_o"),
                        z_ps.tile([128, 1024], F32, tag="z", name="psum_z"),
                    )
                psum_o, psum_z = obank[(h, b)]
                pt, p2t = pts.pop((h, b, jt)), p2ts.pop((h, b, jt))
                vt = v_sb[:, b * 8 + jt, :]
                first, last = jt == 0, jt == JTS - 1
                for hh in (1, 0):
                    d0, d1 = hh * 64, hh * 64 + 64
                    ch0 = h * 128 + hh * 64
                    for c in range(2):
                        nc.tensor.matmul(
                            psum_z[d0:d1, c * 512:(c + 1) * 512],
                            ones64,
                            pt[:, hh * N + c * 512:hh * N + (c + 1) * 512],
                            start=first,
                            stop=last,
                            skip_group_check=True,
                        )
                    for c in range(2):
                        nc.tensor.matmul(
                            psum_o[d0:d1, c * 512:(c + 1) * 512],
                            vt[:, ch0:ch0 + 64],
                            p2t[:, hh * N + c * 512:hh * N + (c + 1) * 512],
                            start=first,
                            stop=last,
                            skip_group_check=True,
                        )
                if last:
                    # 1/Z^2 on DVE as (psum_o * zinv) * zinv: psum_o is
                    # released after the first multiply, shortening the wait
                    # of the next pair-batch's PV matmuls.
                    psum_o, psum_z = obank.pop((h, b))
                    zinv = z_sb_pool.tile([128, 1024], F32, tag="zinv")
                    otmp = z_sb_pool.tile([128, 1024], BF16, tag="otmp")
                    nc.vector.reciprocal_approx_fast(zinv, psum_z)
                    nc.vector.tensor_mul(out=otmp, in0=psum_o, in1=zinv)
                    nc.vector.tensor_mul(
                        out=aoT_sb[:, h, b * N:(b + 1) * N],
                        in0=otmp,
                        in1=zinv,
                    )

            # global slot pipeline: zpv lags s_exp_sq by LAG slots so the
            # PE queue never blocks on the exp/square of the same slot and
            # pair-batch boundaries stay full.
            LAG = 2
            slots = [
                (h, b, jt)
                for h in range(PAIRS)
                for b in range(B_PC)
                for jt in range(JTS)
            ]
            for k in range(len(slots) + LAG):
                if k < len(slots):
                    s_exp_sq(*slots[k])
                if k >= LAG:
                    zpv(*slots[k - LAG])

        # ---- phase 4: out projection + bias ----
        with (
            tc.tile_pool(name="f_sb", bufs=3) as f_sb,
            tc.tile_pool(name="f_ps", bufs=2, space="PSUM") as f_ps,
        ):
            for mt in range(T // 128):
                ps = f_ps.tile([128, DIM], F32, tag="f")
                for c0, c1 in ((0, 512), (512, 768)):
                    for kt in range(KT):
                        nc.tensor.matmul(
                            ps[:, c0:c1],
                            aoT_sb[:, kt, mt * 128:(mt + 1) * 128],
                            w_out_sb[:, kt, c0:c1],
                            start=(kt == 0),
                            stop=False,
                        )
                    nc.tensor.matmul(
                        ps[:, c0:c1],
                        ones1[0:1, 0:128],
                        b_out_sb[0:1, c0:c1],
                        start=False,
                        stop=True,
                    )
                o_sb = f_sb.tile([128, DIM], F32, tag="fo")
                nc.vector.tensor_copy(out=o_sb, in_=ps)
                nc.sync.dma_start(out[mt * 128:(mt + 1) * 128, :], o_sb)


def _ensure_ntff_hook():
    """Install the NTFF profiling hook that bass_utils expects under axon.

    This agent image's ``antenv`` lacks ``axon_hooks``; replicate the shim
    trn_boot would install, backed by /opt/axon/libaxon_pjrt.so.
    """
    import sys
    import types

    try:
        from antenv.axon_hooks import get_axon_ntff_profile_hook  # noqa: F401

        return
    except ImportError:
        pass
    import antenv

    mod = types.ModuleType("antenv.axon_hooks")
    _hook = [None]
    mod.set_axon_ntff_profile_hook = lambda h: _hook.__setitem__(0, h)
    mod.get_axon_ntff_profile_hook = lambda: _hook[0]
    sys.modules["antenv.axon_hooks"] = mod
    antenv.axon_hooks = mod
    try:
        from trn_agent_boot.trn_boot import _ntff_profile_via_ctypes

        mod.set_axon_ntff_profile_hook(
            _ntff_profile_via_ctypes("/opt/axon/libaxon_pjrt.so")
        )
    except Exception:
        pass


_NC_CACHE = None


def _get_nc():
    global _NC_CACHE
    if _NC_CACHE is None:
        _NC_CACHE = _build_kernel()
    return _NC_CACHE


def kernel(x, w_qkv, w_out, b_out, suppress, _trace=False):
    global LAST_RESULTS
    x = np.asarray(x, dtype=np.float32)
    w_qkv = np.asarray(w_qkv, dtype=np.float32)
    w_out_np = np.asarray(w_out, dtype=np.float32)
    b_out_np = np.asarray(b_out, dtype=np.float32)
    c = float(np.asarray(suppress)) / (1.0 + 1e-6)

    bf = ml_dtypes.bfloat16
    w_qk_f = np.ascontiguousarray(w_qkv[:, : 2 * DIM]).copy()
    w_qk_f[:, :DIM] *= SCALE  # fold softmax scale into Q so Exp runs scale-free
    w_qk_b = w_qk_f.astype(bf)
    w_v_b = np.ascontiguousarray(w_qkv[:, 2 * DIM:] * c).astype(bf)
    w_out_b = w_out_np.astype(bf)
    b_out_b = b_out_np.reshape(1, DIM).astype(bf)

    nc = _get_nc()
    in_maps = []
    for core in range(N_CORES):
        xs = x[core * B_PC:(core + 1) * B_PC].reshape(T, DIM)
        xT_b = np.ascontiguousarray(xs.T).astype(bf)
        in_maps.append(
            {
                "xT": xT_b,
                "w_qk": w_qk_b,
                "w_v": w_v_b,
                "w_out": w_out_b,
                "b_out": b_out_b,
            }
        )

    if _trace:
        _ensure_ntff_hook()
    res = run_bass_kernel_spmd(
        nc, in_maps, core_ids=list(range(N_CORES)), trace=_trace
    )
    LAST_RESULTS = res
    outs = [res.results[cc]["out"].reshape(B_PC, N, DIM) for cc in range(N_CORES)]
    return np.concatenate(outs, axis=0)



# revision 16
# speedup vs baseline: 1.0344x; 1.0172x over previous
"""Trainium2 Bass kernel for nn_Attention_5609227288590 (sparse_attention).

Math: the reference's suppress branch with THRES=1.0 has an all-True mask
(every attn value <= its row max), so it reduces exactly to

    attn' = suppress * attn^2 / (rowsum(attn) + 1e-6)

with rowsum(attn) == 1 up to fp rounding of the softmax itself.  Writing
P = exp(S) (no max subtraction needed: |S| <= ~4 for this distribution),
Z_i = sum_j P_ij:

    out_head[i, :] = c * (P∘P) @ V / Z_i^2 ,   c = suppress / (1 + 1e-6)

Per-core layout (data-parallel over batch, 2 batches/core):
  - qkT (channels x tokens) computed with w_qkv as stationary, x^T as moving
    -> Q^T/K^T land head-pair-stacked on partitions for the row-packed
    S^T = K^T.T @ Q^T matmuls (contraction d=64, 2 heads share the PE array).
    The softmax scale is folded into the Q weights host-side so Exp runs
    with the default scale.
  - V computed in (tokens x channels) layout -> V slices are direct lhsT for
    the PV matmul; P2^T is the moving operand (contraction j on partitions).
  - Z by ones[128,64]-stationary matmuls col-packed 2 heads/bank, giving Z
    broadcast across 64 partitions, matching the PV psum layout.
  - Exp(S) is the ONLY ScalarE work (the per-jt critical path, ~2.2us/jt);
    P^2 is one fused [128,2048] DVE square per jt, and 1/Z^2 is DVE
    reciprocal_approx_fast + two multiplies, ordered so psum_z/psum_o
    release as early as possible for the next pair-batch.
  - Attention runs as a single global slot pipeline over (pair, batch, jt);
    Z/PV matmuls lag the S/Exp/square stream by LAG slots (deep pt/p2t
    pools) so the in-order PE queue never blocks the exp cadence, including
    across pair-batch boundaries.
  - attn_outT (channels x tokens) feeds out-proj as lhsT directly; bias is
    added via a K=1 matmul with an all-ones stationary.
  - Input DMAs are ordered (first QK weight block, then x chunks c-major,
    then remaining weights) so the first projection matmul issues ~3us in.
"""

import numpy as np
import ml_dtypes

import concourse.bass as bass
import concourse.mybir as mybir
import concourse.tile as tile
from concourse import bacc
from concourse.bass_utils import run_bass_kernel_spmd

BF16 = mybir.dt.bfloat16
F32 = mybir.dt.float32
AF = mybir.ActivationFunctionType

N_CORES = 8
B = 16
N = 1024
DIM = 768
HEADS = 12
DH = 64
B_PC = B // N_CORES          # 2 batches per core
T = B_PC * N                 # 2048 tokens per core
PAIRS = HEADS // 2           # 6 head pairs
KT = DIM // 128              # 6 contraction tiles for projections
SCALE = DH ** -0.5           # 0.125
JTS = N // 128               # 8 j-tiles per attention step

LAST_RESULTS = None  # BassKernelResults of the last run (for test.py)


def _patch_act_tables():
    """Pin all activations to the natural_log_exp_and_others table set.

    The kernel only uses Exp and Ln.  Left alone, bacc assigns Exp and Ln to
    different sets and the inner loop thrashes ACT_TABLE_LOAD (~2.7us each).
    Emptying every other set (order preserved, so set ids stay valid) forces
    one load total.
    """
    import concourse.hw_specs as hw_specs

    if getattr(bacc, "_act_tables_patched", False):
        return
    orig = hw_specs.get_activation_tables

    def patched(module_arch):
        tabs = orig(module_arch)
        return {
            name: (funcs if name == "natural_log_exp_and_others" else set())
            for name, funcs in tabs.items()
        }

    bacc.get_activation_tables = patched
    bacc._act_tables_patched = True


def _build_kernel():
    _patch_act_tables()
    nc = bacc.Bacc("TRN2", target_bir_lowering=False, debug=False)

    xT = nc.dram_tensor("xT", [DIM, T], BF16, kind="ExternalInput")
    w_qk = nc.dram_tensor("w_qk", [DIM, 2 * DIM], BF16, kind="ExternalInput")
    w_v = nc.dram_tensor("w_v", [DIM, DIM], BF16, kind="ExternalInput")
    w_out = nc.dram_tensor("w_out", [DIM, DIM], BF16, kind="ExternalInput")
    b_out = nc.dram_tensor("b_out", [1, DIM], BF16, kind="ExternalInput")
    out = nc.dram_tensor("out", [T, DIM], F32, kind="ExternalOutput")

    with tile.TileContext(nc) as tc:
        _body(nc, tc, xT, w_qk, w_v, w_out, b_out, out)
    nc.compile()
    return nc


def _body(nc, tc, xT, w_qk, w_v, w_out, b_out, out):
    from contextlib import ExitStack

    ctx = ExitStack()
    with ctx:
        singles = ctx.enter_context(tc.tile_pool(name="singles", bufs=1))

        # ---- persistent SBUF tensors ----
        w_qk_sb = singles.tile([128, KT, 2 * DIM], BF16)
        w_v_sb = singles.tile([128, KT, DIM], BF16)
        w_out_sb = singles.tile([128, KT, DIM], BF16)
        b_out_sb = singles.tile([1, DIM], BF16)
        ones64 = singles.tile([128, DH], BF16)
        ones1 = singles.tile([1, 128], BF16)
        qkT_sb = singles.tile([128, 2 * PAIRS, T], BF16)   # tiles 0-5 Q, 6-11 K
        v_sb = singles.tile([128, T // 128, DIM], BF16)    # [t, c] layout
        aoT_sb = singles.tile([128, KT, T], BF16)          # attn-outT stacked

        w_qk_r = w_qk.rearrange("(ko p) c -> p ko c", p=128)
        w_v_r = w_v.rearrange("(ko p) c -> p ko c", p=128)
        xt_r = xT.rearrange("(ko p) t -> p ko t", p=128)

        # DMA order: first QK weight chunk, then x (c-major so the c-outer
        # projection loop can start as chunks land), then remaining weights.
        nc.sync.dma_start(w_qk_sb[:, :, 0:128], w_qk_r[:, :, 0:128])

        # ---- phase 1+2: projections ----
        with (
            tc.tile_pool(name="xt_pool", bufs=1) as xt_pool,
            tc.tile_pool(name="proj_ps", bufs=2, space="PSUM") as proj_ps,
        ):
            xt_sb = xt_pool.tile([128, KT, T], BF16)
            for c in range(T // 512):
                for kt in range(KT):
                    nc.sync.dma_start(
                        xt_sb[:, kt, c * 512:(c + 1) * 512],
                        xt_r[:, kt, c * 512:(c + 1) * 512],
                    )
            for mt in range(1, 2 * PAIRS):
                nc.sync.dma_start(
                    w_qk_sb[:, :, mt * 128:(mt + 1) * 128],
                    w_qk_r[:, :, mt * 128:(mt + 1) * 128],
                )
            for kt in range(KT):
                nc.sync.dma_start(w_v_sb[:, kt], w_v_r[:, kt])
            nc.sync.dma_start(
                w_out_sb, w_out.rearrange("(ko p) c -> p ko c", p=128)
            )
            nc.sync.dma_start(b_out_sb, b_out[:, :])
            nc.any.memset(ones64, 1.0)
            nc.any.memset(ones1, 1.0)

            for mt in range(2 * PAIRS):
                ps = proj_ps.tile([128, T], F32, tag="proj")
                for c in range(T // 512):
                    for kt in range(KT):
                        nc.tensor.matmul(
                            ps[:, c * 512:(c + 1) * 512],
                            w_qk_sb[:, kt, mt * 128:(mt + 1) * 128],
                            xt_sb[:, kt, c * 512:(c + 1) * 512],
                            start=(kt == 0),
                            stop=(kt == KT - 1),
                        )
                nc.vector.tensor_copy(out=qkT_sb[:, mt, :], in_=ps)

            for mt in range(T // 128):
                ps = proj_ps.tile([128, DIM], F32, tag="proj")
                for kt in range(KT):
                    for c0, c1 in ((0, 512), (512, 768)):
                        nc.tensor.matmul(
                            ps[:, c0:c1],
                            xt_sb[:, kt, mt * 128:(mt + 1) * 128],
                            w_v_sb[:, kt, c0:c1],
                            start=(kt == 0),
                            stop=(kt == KT - 1),
                        )
                nc.vector.tensor_copy(out=v_sb[:, mt, :], in_=ps)

        # ---- phase 3: attention, per (head pair, batch) ----
        with (
            tc.tile_pool(name="pt_pool", bufs=8) as pt_pool,
            tc.tile_pool(name="p2t_pool", bufs=8) as p2t_pool,
            tc.tile_pool(name="z_sb_pool", bufs=1) as z_sb_pool,
            tc.tile_pool(name="s_ps", bufs=2, space="PSUM") as s_ps,
            tc.tile_pool(name="o_ps", bufs=1, space="PSUM") as o_ps,
            tc.tile_pool(name="z_ps", bufs=1, space="PSUM") as z_ps,
        ):
            pts, p2ts = {}, {}
            obank = {}  # (h, b) -> (psum_o, psum_z)

            def s_exp_sq(h, b, jt):
                t0 = b * N
                qT = qkT_sb[:, h, t0:t0 + N]
                kT_ = qkT_sb[:, PAIRS + h, t0:t0 + N]
                pt = pt_pool.tile([128, 2 * N], BF16, tag="pt")
                p2t = p2t_pool.tile([128, 2 * N], BF16, tag="p2t")
                pts[(h, b, jt)], p2ts[(h, b, jt)] = pt, p2t
                with tc.high_priority(offset=90):
                    # hh-grouped: h1's matmuls launch right after the prior
                    # ACT(h1) frees its psum tile and hide under ACT(h0)
                    for hh in (1, 0):
                        d0, d1 = hh * 64, hh * 64 + 64
                        ps = s_ps.tile([128, N], F32, tag="s", name="psS")
                        for c in range(2):
                            nc.tensor.matmul(
                                ps[:, c * 512:(c + 1) * 512],
                                kT_[d0:d1, jt * 128:(jt + 1) * 128],
                                qT[d0:d1, c * 512:(c + 1) * 512],
                                start=True,
                                stop=True,
                            )
                        nc.scalar.activation(
                            pt[:, hh * N:(hh + 1) * N], ps, AF.Exp,
                        )
                    # one fused square for both head halves (DVE 2x bf16)
                    nc.vector.tensor_mul(out=p2t, in0=pt, in1=pt)

            def zpv(h, b, jt):
                # skip_group_check: the sim's global zero-region check
                # mishandles base_partition!=0; col-split groups are
                # HW-safe (verified by direct probe).
                if jt == 0:
                    obank[(h, b)] = (
                        o_ps.tile([128, 1024], F32, tag="o", name="psum_o"),
                        z_ps.tile([128, 1024], F32, tag="z", name="psum_z"),
                    )
                psum_o, psum_z = obank[(h, b)]
                pt, p2t = pts.pop((h, b, jt)), p2ts.pop((h, b, jt))
                vt = v_sb[:, b * 8 + jt, :]
                first, last = jt == 0, jt == JTS - 1
                for hh in (1, 0):
                    d0, d1 = hh * 64, hh * 64 + 64
                    ch0 = h * 128 + hh * 64
                    for c in range(2):
                        nc.tensor.matmul(
                            psum_z[d0:d1, c * 512:(c + 1) * 512],
                            ones64,
                            pt[:, hh * N + c * 512:hh * N + (c + 1) * 512],
                            start=first,
                            stop=last,
                            skip_group_check=True,
                        )
                    for c in range(2):
                        nc.tensor.matmul(
                            psum_o[d0:d1, c * 512:(c + 1) * 512],
                            vt[:, ch0:ch0 + 64],
                            p2t[:, hh * N + c * 512:hh * N + (c + 1) * 512],
                            start=first,
                            stop=last,
                            skip_group_check=True,
                        )
                if last:
                    # 1/Z^2 on DVE as (psum_o * zinv) * zinv: psum_o is
                    # released after the first multiply, shortening the wait
                    # of the next pair-batch's PV matmuls.
                    psum_o, psum_z = obank.pop((h, b))
                    zinv = z_sb_pool.tile([128, 1024], F32, tag="zinv")
                    otmp = z_sb_pool.tile([128, 1024], BF16, tag="otmp")
                    nc.vector.reciprocal_approx_fast(zinv, psum_z)
                    nc.vector.tensor_mul(out=otmp, in0=psum_o, in1=zinv)
                    nc.vector.tensor_mul(
                        out=aoT_sb[:, h, b * N:(b + 1) * N],
                        in0=otmp,
                        in1=zinv,
                    )

            # global slot pipeline: zpv lags s_exp_sq by LAG slots so the
            # PE queue never blocks on the exp/square of the same slot and
            # pair-batch boundaries stay full.
            LAG = 2
            slots = [
                (h, b, jt)
                for h in range(PAIRS)
                for b in range(B_PC)
                for jt in range(JTS)
            ]
            for k in range(len(slots) + LAG):
                if k < len(slots):
                    s_exp_sq(*slots[k])
                if k >= LAG:
                    zpv(*slots[k - LAG])

        # ---- phase 4: out projection + bias ----
        with (
            tc.tile_pool(name="f_sb", bufs=3) as f_sb,
            tc.tile_pool(name="f_ps", bufs=2, space="PSUM") as f_ps,
        ):
            for mt in range(T // 128):
                ps = f_ps.tile([128, DIM], F32, tag="f")
                for c0, c1 in ((0, 512), (512, 768)):
                    for kt in range(KT):
                        nc.tensor.matmul(
                            ps[:, c0:c1],
                            aoT_sb[:, kt, mt * 128:(mt + 1) * 128],
                            w_out_sb[:, kt, c0:c1],
                            start=(kt == 0),
                            stop=False,
                        )
                    nc.tensor.matmul(
                        ps[:, c0:c1],
                        ones1[0:1, 0:128],
                        b_out_sb[0:1, c0:c1],
                        start=False,
                        stop=True,
                    )
                o_sb = f_sb.tile([128, DIM], F32, tag="fo")
                nc.vector.tensor_copy(out=o_sb, in_=ps)
                nc.sync.dma_start(out[mt * 128:(mt + 1) * 128, :], o_sb)


def _ensure_ntff_hook():
    """Install the NTFF profiling hook that bass_utils expects under axon.

    This agent image's ``antenv`` lacks ``axon_hooks``; replicate the shim
    trn_boot would install, backed by /opt/axon/libaxon_pjrt.so.
    """
    import sys
    import types

    try:
        from antenv.axon_hooks import get_axon_ntff_profile_hook  # noqa: F401

        return
    except ImportError:
        pass
    import antenv

    mod = types.ModuleType("antenv.axon_hooks")
    _hook = [None]
    mod.set_axon_ntff_profile_hook = lambda h: _hook.__setitem__(0, h)
    mod.get_axon_ntff_profile_hook = lambda: _hook[0]
    sys.modules["antenv.axon_hooks"] = mod
    antenv.axon_hooks = mod
    try:
        from trn_agent_boot.trn_boot import _ntff_profile_via_ctypes

        mod.set_axon_ntff_profile_hook(
            _ntff_profile_via_ctypes("/opt/axon/libaxon_pjrt.so")
        )
    except Exception:
        pass


_NC_CACHE = None


def _get_nc():
    global _NC_CACHE
    if _NC_CACHE is None:
        _NC_CACHE = _build_kernel()
    return _NC_CACHE


def kernel(x, w_qkv, w_out, b_out, suppress, _trace=False):
    global LAST_RESULTS
    x = np.asarray(x, dtype=np.float32)
    w_qkv = np.asarray(w_qkv, dtype=np.float32)
    w_out_np = np.asarray(w_out, dtype=np.float32)
    b_out_np = np.asarray(b_out, dtype=np.float32)
    c = float(np.asarray(suppress)) / (1.0 + 1e-6)

    bf = ml_dtypes.bfloat16
    w_qk_f = np.ascontiguousarray(w_qkv[:, : 2 * DIM]).copy()
    w_qk_f[:, :DIM] *= SCALE  # fold softmax scale into Q so Exp runs scale-free
    w_qk_b = w_qk_f.astype(bf)
    w_v_b = np.ascontiguousarray(w_qkv[:, 2 * DIM:] * c).astype(bf)
    w_out_b = w_out_np.astype(bf)
    b_out_b = b_out_np.reshape(1, DIM).astype(bf)

    nc = _get_nc()
    in_maps = []
    for core in range(N_CORES):
        xs = x[core * B_PC:(core + 1) * B_PC].reshape(T, DIM)
        xT_b = np.ascontiguousarray(xs.T).astype(bf)
        in_maps.append(
            {
                "xT": xT_b,
                "w_qk": w_qk_b,
                "w_v": w_v_b,
                "w_out": w_out_b,
                "b_out": b_out_b,
            }
        )

    if _trace:
        _ensure_ntff_hook()
    res = run_bass_kernel_spmd(
        nc, in_maps, core_ids=list(range(N_CORES)), trace=_trace
    )
    LAST_RESULTS = res
    outs = [res.results[cc]["out"].reshape(B_PC, N, DIM) for cc in range(N_CORES)]
    return np.concatenate(outs, axis=0)

